# revision 38
# baseline (speedup 1.0000x reference)
"""Trainium2 Bass kernel for an 8-batch image-conditioned decoder layer.

Strategy: pure data-parallel over the batch — core c computes batch element c
end-to-end (causal self-attention, cross-attention over the image tokens, both
layernorms, vocab projection). No collectives.

All matmuls run in bf16 with fp32 PSUM accumulation.  Weights are pre-cast /
pre-tiled on the host into the exact SBUF layouts the TensorEngine consumes
([128 k_inner, k_outer, n]); the vocab projection streams exactly V=32000
columns (62 chunks of 512 + one of 256) from HBM.  The embedding gather and
positional add happen on the host (input prep), shipped both seq-major (x0b,
for residuals) and d-major (x0T, ready for the first projections) so the
TensorEngine starts immediately.  Elementwise work is spread across DVE /
Activation / GpSimd so no single engine serializes the attention phase.
"""

import os
import sys

for _p in ("/opt/trn_rl_repo", "/root/.axon_site/_ro/trn_rl_repo"):
    if os.path.isdir(_p) and _p not in sys.path:
        sys.path.append(_p)

import numpy as np
import ml_dtypes

BF16 = ml_dtypes.bfloat16

# Problem dims (hardcoded per spec)
V, D, DI, S, B, NI = 32000, 1024, 768, 512, 8, 197
EPS = 1e-5
P = 128
ST = S // P          # 4 seq tiles
DT = D // P          # 8 model-dim tiles
DIT = DI // P        # 6 image-dim tiles
NIT = 2              # image tokens: 197 -> 2 partition tiles (128 + 69)
NI_PAD = 256
CN = 512             # vocab chunk width
NFULL = V // CN      # 62 full chunks
CTAIL = V - NFULL * CN   # 256 tail columns
GRP = 2              # full chunks per output strip
NGRP = NFULL // GRP  # 31
N_CORES = 8
SCALE = 1.0 / float(np.sqrt(np.float32(D)))

_CACHE = {}
LAST_RESULTS = None


def _build_program():
    import concourse.bacc as bacc
    import concourse.bass as bass
    import concourse.mybir as mybir
    from concourse.masks import make_identity
    from concourse.tile import TileContext

    f32 = mybir.dt.float32
    bf16 = mybir.dt.bfloat16
    X = mybir.AxisListType.X
    ALU = mybir.AluOpType
    ACT_F = mybir.ActivationFunctionType

    nc = bacc.Bacc("TRN2", target_bir_lowering=False, debug=False,
                   num_devices=N_CORES)

    # ---- I/O ----
    h_x0b = nc.dram_tensor("x0b", [P, ST, D], bf16, kind="ExternalInput")
    h_x0t = nc.dram_tensor("x0t", [P, DT, S], bf16, kind="ExternalInput")
    h_img = nc.dram_tensor("img_t", [P, DIT, NI], bf16, kind="ExternalInput")
    h_wq1 = nc.dram_tensor("wq1", [P, DT, D], bf16, kind="ExternalInput")
    h_wk1 = nc.dram_tensor("wk1", [P, DT, D], bf16, kind="ExternalInput")
    h_wv1 = nc.dram_tensor("wv1", [P, DT, D], bf16, kind="ExternalInput")
    h_wq2 = nc.dram_tensor("wq2", [P, DT, D], bf16, kind="ExternalInput")
    h_wk2 = nc.dram_tensor("wk2", [P, DIT, D], bf16, kind="ExternalInput")
    h_wv2 = nc.dram_tensor("wv2", [P, DIT, D], bf16, kind="ExternalInput")
    h_wp = nc.dram_tensor("wp", [NFULL, P, DT, CN], bf16, kind="ExternalInput")
    h_wpt = nc.dram_tensor("wpt", [P, DT, CTAIL], bf16, kind="ExternalInput")
    h_bqs = nc.dram_tensor("bqs", [P, 4, DT], f32, kind="ExternalInput")
    h_bv1 = nc.dram_tensor("bv1", [1, D], bf16, kind="ExternalInput")
    h_bv2 = nc.dram_tensor("bv2", [1, D], bf16, kind="ExternalInput")
    h_bp = nc.dram_tensor("bp", [V], bf16, kind="ExternalInput")
    h_g1 = nc.dram_tensor("g1", [D], bf16, kind="ExternalInput")
    h_b1 = nc.dram_tensor("b1", [D], bf16, kind="ExternalInput")
    h_g2 = nc.dram_tensor("g2", [D], bf16, kind="ExternalInput")
    h_b2 = nc.dram_tensor("b2", [D], bf16, kind="ExternalInput")
    h_out = nc.dram_tensor("out", [S, V], bf16, kind="ExternalOutput")

    def bcast(handle, n, offset=0):
        ap = handle[:]
        return bass.AP(tensor=ap.tensor, offset=offset, ap=[[0, P], [1, n]])

    with TileContext(nc) as tc:
        import contextlib
        ctx = contextlib.ExitStack()
        with ctx:
            const = ctx.enter_context(tc.tile_pool(name="const", bufs=1))
            xb_p = ctx.enter_context(tc.tile_pool(name="xb", bufs=2))
            xt_p = ctx.enter_context(tc.tile_pool(name="xt", bufs=1))
            qk_p = ctx.enter_context(tc.tile_pool(name="qk", bufs=2))
            v_p = ctx.enter_context(tc.tile_pool(name="vp", bufs=2))
            k2t_p = ctx.enter_context(tc.tile_pool(name="k2t", bufs=1))
            pb_p = ctx.enter_context(tc.tile_pool(name="pb", bufs=4))
            pt_p = ctx.enter_context(tc.tile_pool(name="pt", bufs=1))
            dgl_p = ctx.enter_context(tc.tile_pool(name="dgl", bufs=1))
            xpre_p = ctx.enter_context(tc.tile_pool(name="xpre", bufs=2))
            stat_p = ctx.enter_context(tc.tile_pool(name="stat", bufs=4))
            wts_p = ctx.enter_context(tc.tile_pool(name="wts", bufs=6))
            wp_p = ctx.enter_context(tc.tile_pool(name="wpp", bufs=5))
            bp_p = ctx.enter_context(tc.tile_pool(name="bpp", bufs=2))
            osb_p = ctx.enter_context(tc.tile_pool(name="osb", bufs=5))
            ps = ctx.enter_context(tc.tile_pool(name="ps", bufs=8, space="PSUM"))

            # ---- queue assignments ----
            # All big weights are shipped as 8KB/partition halves so three DMA
            # queues share the early critical bytes and the 6-slot ring never
            # stalls a transfer past its consumer.
            # sync:   x0T, wk1a, wv1a, wp0, wp1, wq2a/b, rest of the wp stream
            # scalar: small biases, wq1a, wk1b, wv1b, bv1/g1/b1, x0b, img,
            #         bv2/g2/b2; later: half the output writes
            # gpsimd: wq1b, wk2a/b, wv2a/b; trimask; bp strips + other outputs
            HD = D // 2
            HK = DT // 2
            x0t_lo = xt_p.tile([P, HK, S], bf16, tag="xtl", name="x0tl")
            nc.sync.dma_start(out=x0t_lo, in_=h_x0t[:, 0:HK, :])
            x0t_hi = xt_p.tile([P, HK, S], bf16, tag="xth", name="x0th")
            nc.scalar.dma_start(out=x0t_hi, in_=h_x0t[:, HK:DT, :])

            def x0T_k(k):
                return (x0t_lo, k) if k < HK else (x0t_hi, k - HK)

            bqall = const.tile([P, 4, DT], f32)
            nc.scalar.dma_start(out=bqall, in_=h_bqs[:])
            QD = D // 4
            wq1q = []
            for qi, eng in enumerate((nc.gpsimd, nc.sync, nc.scalar,
                                      nc.gpsimd)):
                t = wts_p.tile([P, DT, QD], bf16, tag="wts")
                eng.dma_start(out=t, in_=h_wq1[:, :, qi * QD:(qi + 1) * QD])
                wq1q.append(t)
            wk1a = wts_p.tile([P, DT, HD], bf16, tag="wts")
            nc.sync.dma_start(out=wk1a, in_=h_wk1[:, :, 0:HD])
            wk1b = wts_p.tile([P, DT, HD], bf16, tag="wts")
            nc.scalar.dma_start(out=wk1b, in_=h_wk1[:, :, HD:D])
            wv1a = wts_p.tile([P, DT, HD], bf16, tag="wts")
            nc.sync.dma_start(out=wv1a, in_=h_wv1[:, :, 0:HD])
            wv1b = wts_p.tile([P, DT, HD], bf16, tag="wts")
            nc.gpsimd.dma_start(out=wv1b, in_=h_wv1[:, :, HD:D])

            ident = const.tile([P, P], bf16)
            make_identity(nc, ident)
            trimask = const.tile([P, P], f32)
            nc.gpsimd.memset(trimask, 0.0)
            nc.gpsimd.affine_select(
                out=trimask, in_=trimask, compare_op=ALU.is_ge, fill=-1e10,
                base=0, pattern=[[-1, P]], channel_multiplier=1)
            epst = const.tile([P, 1], f32)
            nc.vector.memset(epst, EPS)
            ones_sb = const.tile([P, P], bf16)
            nc.vector.memset(ones_sb, 1.0)
            V2t = v_p.tile([P, NIT, D], bf16, tag="v")
            nc.vector.memset(V2t[:, 1, :], 0.0)

            # single-row bias vectors (consumed via rank-1 ones-row matmuls)
            bv1b = const.tile([P, D], bf16)
            nc.scalar.dma_start(out=bv1b[0:1, :], in_=h_bv1[:])
            bv2b = const.tile([P, D], bf16)
            nc.scalar.dma_start(out=bv2b[0:1, :], in_=h_bv2[:])

            # cross-attn weight halves ride the otherwise-idle gpsimd queue
            wk2a = wts_p.tile([P, DIT, HD], bf16, tag="wts")
            nc.gpsimd.dma_start(out=wk2a, in_=h_wk2[:, :, 0:HD])
            wk2b = wts_p.tile([P, DIT, HD], bf16, tag="wts")
            nc.gpsimd.dma_start(out=wk2b, in_=h_wk2[:, :, HD:D])
            wv2a = wts_p.tile([P, DIT, HD], bf16, tag="wts")
            nc.gpsimd.dma_start(out=wv2a, in_=h_wv2[:, :, 0:HD])
            wv2b = wts_p.tile([P, DIT, HD], bf16, tag="wts")
            nc.gpsimd.dma_start(out=wv2b, in_=h_wv2[:, :, HD:D])

            x0b = xb_p.tile([P, ST, D], bf16, tag="xb", name="x0b")
            nc.scalar.dma_start(out=x0b, in_=h_x0b[:])
            img_sb = const.tile([P, DIT, NI], bf16)
            nc.scalar.dma_start(out=img_sb, in_=h_img[:])

            # layernorm gain/shift broadcasts (needed from ~80us on)
            g1b = const.tile([P, D], bf16)
            b1b = const.tile([P, D], bf16)
            g2b = const.tile([P, D], bf16)
            b2b = const.tile([P, D], bf16)
            for t, h in ((g1b, h_g1), (b1b, h_b1), (g2b, h_g2), (b2b, h_b2)):
                nc.scalar.dma_start(out=t, in_=bcast(h, D))

            # vocab weight stream on sync (wq2 halves slot in mid-stream)
            wp_tiles = []
            for c in range(4):
                wp_sb = wp_p.tile([P, DT, CN], bf16, tag="wp", name=f"wp{c}")
                nc.sync.dma_start(out=wp_sb, in_=h_wp[c])
                wp_tiles.append(wp_sb)
            wpt_sb = const.tile([P, DT, CTAIL], bf16)
            nc.sync.dma_start(out=wpt_sb, in_=h_wpt[:])
            for c in range(4, NFULL):
                wp_sb = wp_p.tile([P, DT, CN], bf16, tag="wp", name=f"wp{c}")
                nc.sync.dma_start(out=wp_sb, in_=h_wp[c])
                wp_tiles.append(wp_sb)

            # ---- projections ----
            def proj_T(w_of_m, b_row, rhs_of_k, name):
                """QT/KT-style: out[P, DT, S] bf16 = (W.T @ x.T) + b, d-partition.
                w_of_m / rhs_of_k map tile indices to (tile, local index)."""
                o = qk_p.tile([P, DT, S], bf16, tag="qk", name=name)
                for m in range(DT):
                    w_sb, mb = w_of_m(m)
                    pm = ps.tile([P, 512], f32, tag="ps", name="pm")
                    for k in range(DT):
                        r_sb, rk = rhs_of_k(k)
                        nc.tensor.matmul(pm,
                                         lhsT=w_sb[:, k, mb:mb + P],
                                         rhs=r_sb[:, rk, :],
                                         start=(k == 0), stop=(k == DT - 1))
                    nc.scalar.activation(out=o[:, m, :], in_=pm,
                                         func=ACT_F.Identity,
                                         bias=bqall[:, b_row, m:m + 1],
                                         scale=1.0)
                return o

            def halved(wa, wb):
                return lambda m: (wa, m * P) if m < 4 else (wb, (m - 4) * P)

            QT = proj_T(lambda m: (wq1q[m // 2], (m % 2) * P), 0, x0T_k, "qt")
            KT = proj_T(halved(wk1a, wk1b), 1, x0T_k, "kt")


            # ---- causal self-attention: scores + softmax (all qt), then AV ----
            # softmax straight out of PSUM: max over the raw scores (safe — the
            # shift cancels in the normalization), mask only the diagonal block.
            Pbs = []
            rinv1 = stat_p.tile([P, ST], f32, tag="rinv")
            for qt in range(ST):
                width = (qt + 1) * P
                pm = ps.tile([P, 512], f32, tag="ps")
                for k in range(DT):
                    nc.tensor.matmul(pm[:, :width],
                                     lhsT=QT[:, k, qt * P:(qt + 1) * P],
                                     rhs=KT[:, k, :width],
                                     start=(k == 0), stop=(k == DT - 1))
                nmax = stat_p.tile([P, 1], f32, tag="nmax")
                nc.vector.reduce_max(nmax, pm[:, :width], axis=X, negate=True)
                diag = dgl_p.tile([P, P], f32, tag="dgl")
                nc.vector.tensor_tensor(out=diag, in0=pm[:, qt * P:width],
                                        in1=trimask, op=ALU.add)
                Pb = pb_p.tile([P, 512], bf16, tag="pb", name=f"pb{qt}")
                rsum = stat_p.tile([P, 1], f32, tag="rsum")
                if qt > 0:
                    rs1 = stat_p.tile([P, 1], f32, tag="rs1")
                    nc.scalar.activation(out=Pb[:, :qt * P], in_=pm[:, :qt * P],
                                         func=ACT_F.Exp, bias=nmax, scale=1.0,
                                         accum_out=rs1)
                    rs2 = stat_p.tile([P, 1], f32, tag="rs2")
                    nc.scalar.activation(out=Pb[:, qt * P:width], in_=diag,
                                         func=ACT_F.Exp, bias=nmax, scale=1.0,
                                         accum_out=rs2)
                    nc.vector.tensor_tensor(out=rsum, in0=rs1, in1=rs2,
                                            op=ALU.add)
                else:
                    nc.scalar.activation(out=Pb[:, :width], in_=diag,
                                         func=ACT_F.Exp, bias=nmax, scale=1.0,
                                         accum_out=rsum)
                nc.vector.reciprocal(out=rinv1[:, qt:qt + 1], in_=rsum)
                Pbs.append(Pb)

            # V projection overlaps the softmax chain above
            Vt = v_p.tile([P, ST, D], bf16, tag="v")
            for a in range(ST):
                for nh in range(2):
                    wv = wv1a if nh == 0 else wv1b
                    pm = ps.tile([P, 512], f32, tag="ps")
                    for k in range(DT):
                        xkt, xk = x0T_k(k)
                        nc.tensor.matmul(
                            pm, lhsT=xkt[:, xk, a * P:(a + 1) * P],
                            rhs=wv[:, k, :],
                            start=(k == 0), stop=False)
                    nc.tensor.matmul(
                        pm, lhsT=ones_sb[0:1, :],
                        rhs=bv1b[0:1, nh * 512:(nh + 1) * 512],
                        start=False, stop=True)
                    nc.scalar.copy(out=Vt[:, a, nh * 512:(nh + 1) * 512],
                                   in_=pm)

            # wq2 triggers emitted here: their ring slots (ex-wv1) are free by
            # the time the Act queue reaches them, so nothing stalls
            wq2a = wts_p.tile([P, DT, HD], bf16, tag="wts")
            nc.scalar.dma_start(out=wq2a, in_=h_wq2[:, :, 0:HD])
            wq2b = wts_p.tile([P, DT, HD], bf16, tag="wts")
            nc.scalar.dma_start(out=wq2b, in_=h_wq2[:, :, HD:D])

            def layernorm(xpre, out_sl, gb, bb):
                """xpre [P, D] f32 -> out_sl [P, D] bf16 (normalized * g + b)."""
                stats = stat_p.tile([P, 2, 6], f32, tag="bnst")
                for sg in range(2):
                    nc.vector.bn_stats(out=stats[:, sg, :],
                                       in_=xpre[:, sg * 512:(sg + 1) * 512])
                mv = stat_p.tile([P, 2], f32, tag="bnmv")
                nc.vector.bn_aggr(out=mv, in_=stats)
                rstd = stat_p.tile([P, 1], f32, tag="rstd")
                nc.scalar.activation(out=rstd, in_=mv[:, 1:2], func=ACT_F.Sqrt,
                                     bias=epst, scale=1.0)
                nc.vector.reciprocal(out=rstd, in_=rstd)
                nmr = stat_p.tile([P, 1], f32, tag="nmr")
                nc.vector.tensor_tensor(out=nmr, in0=mv[:, 0:1], in1=rstd,
                                        op=ALU.mult)
                nc.scalar.mul(nmr, nmr, -1.0)
                nc.scalar.activation(out=xpre, in_=xpre, func=ACT_F.Identity,
                                     bias=nmr, scale=rstd)
                nc.vector.tensor_tensor(out=xpre, in0=xpre, in1=gb,
                                        op=ALU.mult)
                nc.vector.tensor_tensor(out=out_sl, in0=xpre, in1=bb,
                                        op=ALU.add)

            # ---- cross-attn K2/V2 (emitted after AV1 so this independent PE
            # work fills the layernorm-chain window) ----
            K2T = k2t_p.tile([P, DT, NI_PAD], bf16, tag="k2t")

            def emit_k2t():
                for m in range(DT):
                    wk2, mb = (wk2a, m * P) if m < 4 else (wk2b, (m - 4) * P)
                    pm = ps.tile([P, 512], f32, tag="ps")
                    for k in range(DIT):
                        nc.tensor.matmul(pm[:, :NI],
                                         lhsT=wk2[:, k, mb:mb + P],
                                         rhs=img_sb[:, k, :],
                                         start=(k == 0), stop=(k == DIT - 1))
                    nc.scalar.activation(out=K2T[:, m, :NI], in_=pm[:, :NI],
                                         func=ACT_F.Identity,
                                         bias=bqall[:, 3, m:m + 1], scale=1.0)

            def emit_v2t():
              for a in range(NIT):
                  pa = P if a == 0 else NI - P
                  for nh in range(2):
                      wv2 = wv2a if nh == 0 else wv2b
                      pm = ps.tile([P, 512], f32, tag="ps")
                      for k in range(DIT):
                          nc.tensor.matmul(
                              pm[:pa, :], lhsT=img_sb[:, k, a * P:a * P + pa],
                              rhs=wv2[:, k, :],
                              start=(k == 0), stop=False)
                      nc.tensor.matmul(
                          pm[:pa, :], lhsT=ones_sb[0:1, :pa],
                          rhs=bv2b[0:1, nh * 512:(nh + 1) * 512],
                          start=False, stop=True)
                      nc.scalar.copy(out=V2t[:pa, a, nh * 512:(nh + 1) * 512],
                                     in_=pm[:pa, :])

            def transpose_rows(xb_tile, qt, xt, tag):
                """XBAR-transpose row-tile qt of [P, ST, D] into the qt-major
                d-transposed tile xt [P, ST, DT, P] — zero PE/Act cost."""
                nc.scalar.dma_start_transpose(out=xt[:, qt, :, :],
                                               in_=xb_tile[:, qt, :])

            # AV1 + LN1, with row-transposes of x1 staggered one qt behind so
            # the PE never waits on the freshest layernorm
            PT = pt_p.tile([P, ST, S], bf16, tag="pt")
            x1b = xb_p.tile([P, ST, D], bf16, tag="xb")
            x1T = xt_p.tile([P, ST, DT, P], bf16, tag="xt", name="x1t")
            for qt in range(ST):
                for kt in range(qt + 1):
                    tp = ps.tile([P, 512], bf16, tag="ps", name="tp")
                    nc.tensor.transpose(out=tp[:, :P],
                                        in_=Pbs[qt][:, kt * P:(kt + 1) * P],
                                        identity=ident)
                    nc.scalar.copy(out=PT[:, kt, qt * P:(qt + 1) * P],
                                   in_=tp[:, :P])
                xpre = xpre_p.tile([P, D], bf16, tag="xpre")
                for nh in range(2):
                    pm = ps.tile([P, 512], f32, tag="ps")
                    for kt in range(qt + 1):
                        nc.tensor.matmul(pm, lhsT=PT[:, kt, qt * P:(qt + 1) * P],
                                         rhs=Vt[:, kt, nh * 512:(nh + 1) * 512],
                                         start=(kt == 0), stop=(kt == qt))
                    nc.vector.scalar_tensor_tensor(
                        out=xpre[:, nh * 512:(nh + 1) * 512], in0=pm,
                        scalar=rinv1[:, qt:qt + 1],
                        in1=x0b[:, qt, nh * 512:(nh + 1) * 512],
                        op0=ALU.mult, op1=ALU.add)
                if qt > 1:
                    transpose_rows(x1b, qt - 2, x1T, "x1")
                layernorm(xpre, x1b[:, qt, :], g1b, b1b)
            transpose_rows(x1b, ST - 2, x1T, "x1")
            emit_k2t()
            emit_v2t()

            # ---- cross attention: Q2, scores2 + softmax, then AV2 ----
            # Q2 columns for the first three row-tiles keep the PE busy while
            # LN1 of the last row-tile drains; its transpose slots in between.
            Q2T = qk_p.tile([P, DT, S], bf16, tag="qk", name="q2t")

            def q2t_range(s0, s1):
                for m in range(DT):
                    w_sb, mb = (wq2a, m * P) if m < 4 else (wq2b, (m - 4) * P)
                    pm = ps.tile([P, 512], f32, tag="ps", name="pm")
                    for k in range(DT):
                        nc.tensor.matmul(pm[:, :s1 - s0],
                                         lhsT=w_sb[:, k, mb:mb + P],
                                         rhs=x1T[:, s0 // P:s1 // P, k, :],
                                         start=(k == 0), stop=(k == DT - 1))
                    nc.scalar.activation(out=Q2T[:, m, s0:s1],
                                         in_=pm[:, :s1 - s0],
                                         func=ACT_F.Identity,
                                         bias=bqall[:, 2, m:m + 1], scale=1.0)

            q2t_range(0, 3 * P)
            transpose_rows(x1b, ST - 1, x1T, "x1")
            q2t_range(3 * P, S)

            P2bs = []
            rinv2 = stat_p.tile([P, ST], f32, tag="rinv")
            for qt in range(ST):
                pm = ps.tile([P, 512], f32, tag="ps")
                for k in range(DT):
                    nc.tensor.matmul(pm[:, :NI],
                                     lhsT=Q2T[:, k, qt * P:(qt + 1) * P],
                                     rhs=K2T[:, k, :NI],
                                     start=(k == 0), stop=(k == DT - 1))
                nmax = stat_p.tile([P, 1], f32, tag="nmax")
                nc.vector.reduce_max(nmax, pm[:, :NI], axis=X, negate=True)
                P2b = pb_p.tile([P, NI_PAD], bf16, tag="pb", name=f"p2b{qt}")
                nc.vector.memset(P2b[:, NI:], 0.0)
                rsum = stat_p.tile([P, 1], f32, tag="rsum")
                nc.scalar.activation(out=P2b[:, :NI], in_=pm[:, :NI],
                                     func=ACT_F.Exp, bias=nmax, scale=1.0,
                                     accum_out=rsum)
                nc.vector.reciprocal(out=rinv2[:, qt:qt + 1], in_=rsum)
                P2bs.append(P2b)

            PT2 = pt_p.tile([P, NIT, S], bf16, tag="pt")
            x2b = xb_p.tile([P, ST, D], bf16, tag="xb")
            x2T = xt_p.tile([P, ST, DT, P], bf16, tag="xt", name="x2t")
            for qt in range(ST):
                for kt in range(NIT):
                    tp = ps.tile([P, 512], bf16, tag="ps", name="tp")
                    nc.tensor.transpose(out=tp[:, :P],
                                        in_=P2bs[qt][:, kt * P:(kt + 1) * P],
                                        identity=ident)
                    nc.scalar.copy(out=PT2[:, kt, qt * P:(qt + 1) * P],
                                   in_=tp[:, :P])
                xpre = xpre_p.tile([P, D], bf16, tag="xpre")
                for nh in range(2):
                    pm = ps.tile([P, 512], f32, tag="ps")
                    for kt in range(NIT):
                        nc.tensor.matmul(pm, lhsT=PT2[:, kt, qt * P:(qt + 1) * P],
                                         rhs=V2t[:, kt, nh * 512:(nh + 1) * 512],
                                         start=(kt == 0), stop=(kt == NIT - 1))
                    nc.vector.scalar_tensor_tensor(
                        out=xpre[:, nh * 512:(nh + 1) * 512], in0=pm,
                        scalar=rinv2[:, qt:qt + 1],
                        in1=x1b[:, qt, nh * 512:(nh + 1) * 512],
                        op0=ALU.mult, op1=ALU.add)
                if qt > 1:
                    transpose_rows(x2b, qt - 2, x2T, "x2")
                layernorm(xpre, x2b[:, qt, :], g2b, b2b)
            transpose_rows(x2b, ST - 2, x2T, "x2")

            # ---- vocab projection, streamed in CN-column chunks ----
            def vocab_group_mm(chunks, widths, offset, qt_list, state=None):
                gw = sum(widths)
                if state is None:
                    bp_bc = bp_p.tile([P, gw], bf16, tag="bp")
                    nc.gpsimd.dma_start(out=bp_bc,
                                        in_=bcast(h_bp, gw, offset=offset))
                    osb = [osb_p.tile([P, gw], bf16, tag="osb",
                                      name=f"osb_{offset}_{q}")
                           for q in range(ST)]
                else:
                    bp_bc, osb = state
                col = 0
                for wp_sb, w in zip(chunks, widths):
                    for qt in qt_list:
                        pm = ps.tile([P, 512], f32, tag="ps")
                        for k in range(DT):
                            nc.tensor.matmul(
                                pm[:, :w], lhsT=x2T[:, qt, k, :],
                                rhs=wp_sb[:, k, :w],
                                start=(k == 0), stop=(k == DT - 1))
                        nc.vector.tensor_tensor(
                            out=osb[qt][:, col:col + w], in0=pm[:, :w],
                            in1=bp_bc[:, col:col + w], op=ALU.add)
                    col += w
                return bp_bc, osb

            def vocab_group_out(osb, offset, gw):
                for qt in range(ST):
                    out_eng = nc.scalar if qt < 2 else nc.gpsimd
                    out_eng.dma_start(
                        out=h_out[qt * P:(qt + 1) * P, offset:offset + gw],
                        in_=osb[qt])

            # group 0: row-tile 3 deferred until its transpose lands
            g0 = [wp_tiles[0], wp_tiles[1]]
            st0 = vocab_group_mm(g0, [CN, CN], 0, [0])
            vocab_group_mm(g0, [CN, CN], 0, [1], state=st0)
            vocab_group_mm(g0, [CN, CN], 0, [2], state=st0)
            transpose_rows(x2b, ST - 1, x2T, "x2")
            vocab_group_mm(g0, [CN, CN], 0, [3], state=st0)
            vocab_group_out(st0[1], 0, GRP * CN)
            g1 = [wp_tiles[2], wp_tiles[3]]
            st1 = vocab_group_mm(g1, [CN, CN], GRP * CN, [0])
            for q in (1, 2, 3):
                vocab_group_mm(g1, [CN, CN], GRP * CN, [q], state=st1)
            vocab_group_out(st1[1], GRP * CN, GRP * CN)
            # tail strip early so the kernel doesn't end on a ragged group
            _, osbt = vocab_group_mm([wpt_sb], [CTAIL], NFULL * CN,
                                     list(range(ST)))
            vocab_group_out(osbt, NFULL * CN, CTAIL)
            for g in range(2, NGRP):
                _, osb = vocab_group_mm(
                    [wp_tiles[g * GRP], wp_tiles[g * GRP + 1]],
                    [CN, CN], g * GRP * CN, list(range(ST)))
                vocab_group_out(osb, g * GRP * CN, GRP * CN)

    nc.compile()
    return nc


def _tile_sq(w, kt):
    """[K, N] -> [128, K//128, N] contiguous."""
    k, n = w.shape
    assert k == kt * P
    return np.ascontiguousarray(
        w.reshape(kt, P, n).transpose(1, 0, 2)).astype(BF16)


def _prep_inputs(inputs):
    g = lambda name: np.asarray(inputs[name], dtype=np.float32)
    tokens = np.asarray(inputs["tokens"]).astype(np.int64)
    img = g("img_emb")

    # positional encoding (same closed form as the model definition)
    posn = np.arange(S)[:, None].astype(np.float32)
    i = np.arange(0, D, 2).astype(np.float32)
    ang = posn / np.power(10000.0, i / D)
    pos = np.zeros((S, D), dtype=np.float32)
    pos[:, 0::2] = np.sin(ang)
    pos[:, 1::2] = np.cos(ang)

    # embedding gather + positional add on the host (input prep)
    x0 = (g("emb_table")[tokens] + pos[None]).astype(BF16)  # [B, S, D]

    wp = g("Wp")  # [D, V]
    wp_main = np.ascontiguousarray(
        wp[:, :NFULL * CN].reshape(DT, P, NFULL, CN)
        .transpose(2, 1, 0, 3)).astype(BF16)
    wp_tail = _tile_sq(wp[:, NFULL * CN:], DT)
    bp = g("bp").astype(BF16)

    def bias_tiled(b):
        return np.ascontiguousarray(b.reshape(DT, P).T).astype(np.float32)

    shared = {
        "wq1": _tile_sq(g("Wq1") * SCALE, DT),
        "wk1": _tile_sq(g("Wk1"), DT),
        "wv1": _tile_sq(g("Wv1"), DT),
        "wq2": _tile_sq(g("Wq2") * SCALE, DT),
        "wk2": _tile_sq(g("Wk2"), DIT),
        "wv2": _tile_sq(g("Wv2"), DIT),
        "wp": wp_main,
        "wpt": wp_tail,
        "bqs": np.ascontiguousarray(np.stack(
            [bias_tiled(g("bq1") * SCALE), bias_tiled(g("bk1")),
             bias_tiled(g("bq2") * SCALE), bias_tiled(g("bk2"))], axis=1)),
        "bv1": g("bv1").astype(BF16).reshape(1, D),
        "bv2": g("bv2").astype(BF16).reshape(1, D),
        "bp": bp,
        "g1": g("g1").astype(BF16), "b1": g("b1").astype(BF16),
        "g2": g("g2").astype(BF16), "b2": g("b2").astype(BF16),
    }
    in_maps = []
    for c in range(N_CORES):
        m = dict(shared)
        xc = x0[c]  # [S, D] bf16
        m["x0b"] = np.ascontiguousarray(
            xc.reshape(ST, P, D).transpose(1, 0, 2))
        m["x0t"] = np.ascontiguousarray(
            xc.T.reshape(DT, P, S).transpose(1, 0, 2))
        m["img_t"] = np.ascontiguousarray(
            img[c].T.reshape(DIT, P, NI).transpose(1, 0, 2)).astype(BF16)
        in_maps.append(m)
    return in_maps


def _ensure_axon_hooks():
    """bass_utils imports antenv.axon_hooks when BASS_TRACE is set; stub it
    if the module is absent so tracing degrades instead of crashing."""
    try:
        import antenv.axon_hooks  # noqa: F401
    except ImportError:
        import types
        mod = types.ModuleType("antenv.axon_hooks")
        mod.get_axon_ntff_profile_hook = lambda: None
        mod.set_axon_ntff_profile_hook = lambda h: None
        sys.modules["antenv.axon_hooks"] = mod


def kernel(**inputs):
    global LAST_RESULTS
    _ensure_axon_hooks()
    from concourse.bass_utils import run_bass_kernel_spmd

    if "nc" not in _CACHE:
        _CACHE["nc"] = _build_program()
    nc = _CACHE["nc"]

    in_maps = _prep_inputs(inputs)
    res = run_bass_kernel_spmd(nc, in_maps, core_ids=list(range(N_CORES)))
    LAST_RESULTS = res
    out = np.stack([res.results[c]["out"].astype(np.float32)
                    for c in range(N_CORES)])
    return out


# revision 39
# speedup vs baseline: 1.0064x; 1.0064x over previous
"""Trainium2 Bass kernel for an 8-batch image-conditioned decoder layer.

Strategy: pure data-parallel over the batch — core c computes batch element c
end-to-end (causal self-attention, cross-attention over the image tokens, both
layernorms, vocab projection). No collectives.

All matmuls run in bf16 with fp32 PSUM accumulation.  Weights are pre-cast /
pre-tiled on the host into the exact SBUF layouts the TensorEngine consumes
([128 k_inner, k_outer, n]); the vocab projection streams exactly V=32000
columns (62 chunks of 512 + one of 256) from HBM.  The embedding gather and
positional add happen on the host (input prep), shipped both seq-major (x0b,
for residuals) and d-major (x0T, ready for the first projections) so the
TensorEngine starts immediately.  Elementwise work is spread across DVE /
Activation / GpSimd so no single engine serializes the attention phase.
"""

import os
import sys

for _p in ("/opt/trn_rl_repo", "/root/.axon_site/_ro/trn_rl_repo"):
    if os.path.isdir(_p) and _p not in sys.path:
        sys.path.append(_p)

import numpy as np
import ml_dtypes

BF16 = ml_dtypes.bfloat16

# Problem dims (hardcoded per spec)
V, D, DI, S, B, NI = 32000, 1024, 768, 512, 8, 197
EPS = 1e-5
P = 128
ST = S // P          # 4 seq tiles
DT = D // P          # 8 model-dim tiles
DIT = DI // P        # 6 image-dim tiles
NIT = 2              # image tokens: 197 -> 2 partition tiles (128 + 69)
NI_PAD = 256
CN = 512             # vocab chunk width
NFULL = V // CN      # 62 full chunks
CTAIL = V - NFULL * CN   # 256 tail columns
GRP = 2              # full chunks per output strip
NGRP = NFULL // GRP  # 31
N_CORES = 8
SCALE = 1.0 / float(np.sqrt(np.float32(D)))

_CACHE = {}
LAST_RESULTS = None


def _build_program():
    import concourse.bacc as bacc
    import concourse.bass as bass
    import concourse.mybir as mybir
    from concourse.masks import make_identity
    from concourse.tile import TileContext

    f32 = mybir.dt.float32
    bf16 = mybir.dt.bfloat16
    X = mybir.AxisListType.X
    ALU = mybir.AluOpType
    ACT_F = mybir.ActivationFunctionType

    nc = bacc.Bacc("TRN2", target_bir_lowering=False, debug=False,
                   num_devices=N_CORES)

    # ---- I/O ----
    h_x0b = nc.dram_tensor("x0b", [P, ST, D], bf16, kind="ExternalInput")
    h_x0t = nc.dram_tensor("x0t", [P, DT, S], bf16, kind="ExternalInput")
    h_img = nc.dram_tensor("img_t", [P, DIT, NI], bf16, kind="ExternalInput")
    h_wq1 = nc.dram_tensor("wq1", [P, DT, D], bf16, kind="ExternalInput")
    h_wk1 = nc.dram_tensor("wk1", [P, DT, D], bf16, kind="ExternalInput")
    h_wv1 = nc.dram_tensor("wv1", [P, DT, D], bf16, kind="ExternalInput")
    h_wq2 = nc.dram_tensor("wq2", [P, DT, D], bf16, kind="ExternalInput")
    h_wk2 = nc.dram_tensor("wk2", [P, DIT, D], bf16, kind="ExternalInput")
    h_wv2 = nc.dram_tensor("wv2", [P, DIT, D], bf16, kind="ExternalInput")
    h_wp = nc.dram_tensor("wp", [NFULL, P, DT, CN], bf16, kind="ExternalInput")
    h_wpt = nc.dram_tensor("wpt", [P, DT, CTAIL], bf16, kind="ExternalInput")
    h_bqs = nc.dram_tensor("bqs", [P, 4, DT], f32, kind="ExternalInput")
    h_bv1 = nc.dram_tensor("bv1", [1, D], bf16, kind="ExternalInput")
    h_bv2 = nc.dram_tensor("bv2", [1, D], bf16, kind="ExternalInput")
    h_bp = nc.dram_tensor("bp", [V], bf16, kind="ExternalInput")
    h_g1 = nc.dram_tensor("g1", [D], bf16, kind="ExternalInput")
    h_b1 = nc.dram_tensor("b1", [D], bf16, kind="ExternalInput")
    h_g2 = nc.dram_tensor("g2", [D], bf16, kind="ExternalInput")
    h_b2 = nc.dram_tensor("b2", [D], bf16, kind="ExternalInput")
    h_out = nc.dram_tensor("out", [S, V], bf16, kind="ExternalOutput")

    def bcast(handle, n, offset=0):
        ap = handle[:]
        return bass.AP(tensor=ap.tensor, offset=offset, ap=[[0, P], [1, n]])

    with TileContext(nc) as tc:
        import contextlib
        ctx = contextlib.ExitStack()
        with ctx:
            const = ctx.enter_context(tc.tile_pool(name="const", bufs=1))
            xb_p = ctx.enter_context(tc.tile_pool(name="xb", bufs=2))
            xt_p = ctx.enter_context(tc.tile_pool(name="xt", bufs=1))
            qk_p = ctx.enter_context(tc.tile_pool(name="qk", bufs=2))
            v_p = ctx.enter_context(tc.tile_pool(name="vp", bufs=2))
            k2t_p = ctx.enter_context(tc.tile_pool(name="k2t", bufs=1))
            pb_p = ctx.enter_context(tc.tile_pool(name="pb", bufs=4))
            pt_p = ctx.enter_context(tc.tile_pool(name="pt", bufs=1))
            dgl_p = ctx.enter_context(tc.tile_pool(name="dgl", bufs=1))
            xpre_p = ctx.enter_context(tc.tile_pool(name="xpre", bufs=2))
            stat_p = ctx.enter_context(tc.tile_pool(name="stat", bufs=4))
            wts_p = ctx.enter_context(tc.tile_pool(name="wts", bufs=6))
            wp_p = ctx.enter_context(tc.tile_pool(name="wpp", bufs=5))
            bp_p = ctx.enter_context(tc.tile_pool(name="bpp", bufs=2))
            osb_p = ctx.enter_context(tc.tile_pool(name="osb", bufs=5))
            ps = ctx.enter_context(tc.tile_pool(name="ps", bufs=8, space="PSUM"))

            # ---- queue assignments ----
            # All big weights are shipped as 8KB/partition halves so three DMA
            # queues share the early critical bytes and the 6-slot ring never
            # stalls a transfer past its consumer.
            # sync:   x0T, wk1a, wv1a, wp0, wp1, wq2a/b, rest of the wp stream
            # scalar: small biases, wq1a, wk1b, wv1b, bv1/g1/b1, x0b, img,
            #         bv2/g2/b2; later: half the output writes
            # gpsimd: wq1b, wk2a/b, wv2a/b; trimask; bp strips + other outputs
            HD = D // 2
            HK = DT // 2
            x0t_lo = xt_p.tile([P, HK, S], bf16, tag="xtl", name="x0tl")
            nc.sync.dma_start(out=x0t_lo, in_=h_x0t[:, 0:HK, :])
            x0t_hi = xt_p.tile([P, HK, S], bf16, tag="xth", name="x0th")
            nc.scalar.dma_start(out=x0t_hi, in_=h_x0t[:, HK:DT, :])

            def x0T_k(k):
                return (x0t_lo, k) if k < HK else (x0t_hi, k - HK)

            bqall = const.tile([P, 4, DT], f32)
            nc.scalar.dma_start(out=bqall, in_=h_bqs[:])
            QD = D // 4
            wq1q = []
            for qi, eng in enumerate((nc.gpsimd, nc.sync, nc.scalar,
                                      nc.gpsimd)):
                t = wts_p.tile([P, DT, QD], bf16, tag="wts")
                eng.dma_start(out=t, in_=h_wq1[:, :, qi * QD:(qi + 1) * QD])
                wq1q.append(t)
            wk1a = wts_p.tile([P, DT, HD], bf16, tag="wts")
            nc.sync.dma_start(out=wk1a, in_=h_wk1[:, :, 0:HD])
            wk1b = wts_p.tile([P, DT, HD], bf16, tag="wts")
            nc.scalar.dma_start(out=wk1b, in_=h_wk1[:, :, HD:D])
            wv1a = wts_p.tile([P, DT, HD], bf16, tag="wts")
            nc.sync.dma_start(out=wv1a, in_=h_wv1[:, :, 0:HD])
            wv1b = wts_p.tile([P, DT, HD], bf16, tag="wts")
            nc.gpsimd.dma_start(out=wv1b, in_=h_wv1[:, :, HD:D])

            ident = const.tile([P, P], bf16)
            make_identity(nc, ident)
            trimask = const.tile([P, P], f32)
            nc.gpsimd.memset(trimask, 0.0)
            nc.gpsimd.affine_select(
                out=trimask, in_=trimask, compare_op=ALU.is_ge, fill=-1e10,
                base=0, pattern=[[-1, P]], channel_multiplier=1)
            epst = const.tile([P, 1], f32)
            nc.vector.memset(epst, EPS)
            ones_sb = const.tile([P, P], bf16)
            nc.vector.memset(ones_sb, 1.0)
            V2t = v_p.tile([P, NIT, D], bf16, tag="v")
            nc.vector.memset(V2t[:, 1, :], 0.0)

            # single-row bias vectors (consumed via rank-1 ones-row matmuls)
            bv1b = const.tile([P, D], bf16)
            nc.scalar.dma_start(out=bv1b[0:1, :], in_=h_bv1[:])
            bv2b = const.tile([P, D], bf16)
            nc.scalar.dma_start(out=bv2b[0:1, :], in_=h_bv2[:])

            # cross-attn weight halves ride the otherwise-idle gpsimd queue
            wk2a = wts_p.tile([P, DIT, HD], bf16, tag="wts")
            nc.gpsimd.dma_start(out=wk2a, in_=h_wk2[:, :, 0:HD])
            wk2b = wts_p.tile([P, DIT, HD], bf16, tag="wts")
            nc.gpsimd.dma_start(out=wk2b, in_=h_wk2[:, :, HD:D])
            wv2a = wts_p.tile([P, DIT, HD], bf16, tag="wts")
            nc.gpsimd.dma_start(out=wv2a, in_=h_wv2[:, :, 0:HD])
            wv2b = wts_p.tile([P, DIT, HD], bf16, tag="wts")
            nc.gpsimd.dma_start(out=wv2b, in_=h_wv2[:, :, HD:D])

            x0b = xb_p.tile([P, ST, D], bf16, tag="xb", name="x0b")
            nc.scalar.dma_start(out=x0b, in_=h_x0b[:])
            img_sb = const.tile([P, DIT, NI], bf16)
            nc.scalar.dma_start(out=img_sb, in_=h_img[:])

            # layernorm gain/shift broadcasts (needed from ~80us on)
            g1b = const.tile([P, D], bf16)
            b1b = const.tile([P, D], bf16)
            g2b = const.tile([P, D], bf16)
            b2b = const.tile([P, D], bf16)
            for t, h in ((g1b, h_g1), (b1b, h_b1), (g2b, h_g2), (b2b, h_b2)):
                nc.scalar.dma_start(out=t, in_=bcast(h, D))

            # vocab weight stream on sync (wq2 halves slot in mid-stream)
            wp_tiles = []
            for c in range(4):
                wp_sb = wp_p.tile([P, DT, CN], bf16, tag="wp", name=f"wp{c}")
                nc.sync.dma_start(out=wp_sb, in_=h_wp[c])
                wp_tiles.append(wp_sb)
            wpt_sb = const.tile([P, DT, CTAIL], bf16)
            nc.sync.dma_start(out=wpt_sb, in_=h_wpt[:])
            for c in range(4, NFULL):
                wp_sb = wp_p.tile([P, DT, CN], bf16, tag="wp", name=f"wp{c}")
                nc.sync.dma_start(out=wp_sb, in_=h_wp[c])
                wp_tiles.append(wp_sb)

            # ---- projections ----
            def proj_T(w_of_m, b_row, rhs_of_k, name):
                """QT/KT-style: out[P, DT, S] bf16 = (W.T @ x.T) + b, d-partition.
                w_of_m / rhs_of_k map tile indices to (tile, local index)."""
                o = qk_p.tile([P, DT, S], bf16, tag="qk", name=name)
                for m in range(DT):
                    w_sb, mb = w_of_m(m)
                    pm = ps.tile([P, 512], f32, tag="ps", name="pm")
                    for k in range(DT):
                        r_sb, rk = rhs_of_k(k)
                        nc.tensor.matmul(pm,
                                         lhsT=w_sb[:, k, mb:mb + P],
                                         rhs=r_sb[:, rk, :],
                                         start=(k == 0), stop=(k == DT - 1))
                    nc.scalar.activation(out=o[:, m, :], in_=pm,
                                         func=ACT_F.Identity,
                                         bias=bqall[:, b_row, m:m + 1],
                                         scale=1.0)
                return o

            def halved(wa, wb):
                return lambda m: (wa, m * P) if m < 4 else (wb, (m - 4) * P)

            QT = proj_T(lambda m: (wq1q[m // 2], (m % 2) * P), 0, x0T_k, "qt")
            KT = proj_T(halved(wk1a, wk1b), 1, x0T_k, "kt")


            # ---- causal self-attention: scores + softmax (all qt), then AV ----
            # softmax straight out of PSUM: max over the raw scores (safe — the
            # shift cancels in the normalization), mask only the diagonal block.
            Pbs = []
            rinv1 = stat_p.tile([P, ST], f32, tag="rinv")
            for qt in range(ST):
                width = (qt + 1) * P
                pm = ps.tile([P, 512], f32, tag="ps")
                for k in range(DT):
                    nc.tensor.matmul(pm[:, :width],
                                     lhsT=QT[:, k, qt * P:(qt + 1) * P],
                                     rhs=KT[:, k, :width],
                                     start=(k == 0), stop=(k == DT - 1))
                nmax = stat_p.tile([P, 1], f32, tag="nmax")
                nc.vector.reduce_max(nmax, pm[:, :width], axis=X, negate=True)
                diag = dgl_p.tile([P, P], f32, tag="dgl")
                nc.vector.tensor_tensor(out=diag, in0=pm[:, qt * P:width],
                                        in1=trimask, op=ALU.add)
                Pb = pb_p.tile([P, 512], bf16, tag="pb", name=f"pb{qt}")
                rsum = stat_p.tile([P, 1], f32, tag="rsum")
                if qt > 0:
                    rs1 = stat_p.tile([P, 1], f32, tag="rs1")
                    nc.scalar.activation(out=Pb[:, :qt * P], in_=pm[:, :qt * P],
                                         func=ACT_F.Exp, bias=nmax, scale=1.0,
                                         accum_out=rs1)
                    rs2 = stat_p.tile([P, 1], f32, tag="rs2")
                    nc.scalar.activation(out=Pb[:, qt * P:width], in_=diag,
                                         func=ACT_F.Exp, bias=nmax, scale=1.0,
                                         accum_out=rs2)
                    nc.vector.tensor_tensor(out=rsum, in0=rs1, in1=rs2,
                                            op=ALU.add)
                else:
                    nc.scalar.activation(out=Pb[:, :width], in_=diag,
                                         func=ACT_F.Exp, bias=nmax, scale=1.0,
                                         accum_out=rsum)
                nc.vector.reciprocal(out=rinv1[:, qt:qt + 1], in_=rsum)
                Pbs.append(Pb)

            # V projection overlaps the softmax chain above
            Vt = v_p.tile([P, ST, D], bf16, tag="v")
            for a in range(ST):
                for nh in range(2):
                    wv = wv1a if nh == 0 else wv1b
                    pm = ps.tile([P, 512], f32, tag="ps")
                    for k in range(DT):
                        xkt, xk = x0T_k(k)
                        nc.tensor.matmul(
                            pm, lhsT=xkt[:, xk, a * P:(a + 1) * P],
                            rhs=wv[:, k, :],
                            start=(k == 0), stop=False)
                    nc.tensor.matmul(
                        pm, lhsT=ones_sb[0:1, :],
                        rhs=bv1b[0:1, nh * 512:(nh + 1) * 512],
                        start=False, stop=True)
                    nc.scalar.copy(out=Vt[:, a, nh * 512:(nh + 1) * 512],
                                   in_=pm)

            # wq2 triggers emitted here: their ring slots (ex-wv1) are free by
            # the time the Act queue reaches them, so nothing stalls
            wq2a = wts_p.tile([P, DT, HD], bf16, tag="wts")
            nc.scalar.dma_start(out=wq2a, in_=h_wq2[:, :, 0:HD])
            wq2b = wts_p.tile([P, DT, HD], bf16, tag="wts")
            nc.scalar.dma_start(out=wq2b, in_=h_wq2[:, :, HD:D])

            def layernorm(xpre, out_sl, gb, bb):
                """xpre [P, D] f32 -> out_sl [P, D] bf16 (normalized * g + b)."""
                stats = stat_p.tile([P, 2, 6], f32, tag="bnst")
                for sg in range(2):
                    nc.vector.bn_stats(out=stats[:, sg, :],
                                       in_=xpre[:, sg * 512:(sg + 1) * 512])
                mv = stat_p.tile([P, 2], f32, tag="bnmv")
                nc.vector.bn_aggr(out=mv, in_=stats)
                rstd = stat_p.tile([P, 1], f32, tag="rstd")
                nc.scalar.activation(out=rstd, in_=mv[:, 1:2], func=ACT_F.Sqrt,
                                     bias=epst, scale=1.0)
                nc.vector.reciprocal(out=rstd, in_=rstd)
                nmr = stat_p.tile([P, 1], f32, tag="nmr")
                nc.vector.tensor_tensor(out=nmr, in0=mv[:, 0:1], in1=rstd,
                                        op=ALU.mult)
                nc.scalar.mul(nmr, nmr, -1.0)
                nc.scalar.activation(out=xpre, in_=xpre, func=ACT_F.Identity,
                                     bias=nmr, scale=rstd)
                nc.vector.tensor_tensor(out=xpre, in0=xpre, in1=gb,
                                        op=ALU.mult)
                nc.vector.tensor_tensor(out=out_sl, in0=xpre, in1=bb,
                                        op=ALU.add)

            # ---- cross-attn K2/V2 (emitted after AV1 so this independent PE
            # work fills the layernorm-chain window) ----
            K2T = k2t_p.tile([P, DT, NI_PAD], bf16, tag="k2t")

            def emit_k2t():
                for m in range(DT):
                    wk2, mb = (wk2a, m * P) if m < 4 else (wk2b, (m - 4) * P)
                    pm = ps.tile([P, 512], f32, tag="ps")
                    for k in range(DIT):
                        nc.tensor.matmul(pm[:, :NI],
                                         lhsT=wk2[:, k, mb:mb + P],
                                         rhs=img_sb[:, k, :],
                                         start=(k == 0), stop=(k == DIT - 1))
                    nc.scalar.activation(out=K2T[:, m, :NI], in_=pm[:, :NI],
                                         func=ACT_F.Identity,
                                         bias=bqall[:, 3, m:m + 1], scale=1.0)

            def emit_v2t():
              for a in range(NIT):
                  pa = P if a == 0 else NI - P
                  for nh in range(2):
                      wv2 = wv2a if nh == 0 else wv2b
                      pm = ps.tile([P, 512], f32, tag="ps")
                      for k in range(DIT):
                          nc.tensor.matmul(
                              pm[:pa, :], lhsT=img_sb[:, k, a * P:a * P + pa],
                              rhs=wv2[:, k, :],
                              start=(k == 0), stop=False)
                      nc.tensor.matmul(
                          pm[:pa, :], lhsT=ones_sb[0:1, :pa],
                          rhs=bv2b[0:1, nh * 512:(nh + 1) * 512],
                          start=False, stop=True)
                      nc.scalar.copy(out=V2t[:pa, a, nh * 512:(nh + 1) * 512],
                                     in_=pm[:pa, :])

            def transpose_rows(xb_tile, qt, xt, tag):
                """XBAR-transpose row-tile qt of [P, ST, D] into the qt-major
                d-transposed tile xt [P, ST, DT, P] — zero PE/Act cost."""
                nc.scalar.dma_start_transpose(out=xt[:, qt, :, :],
                                               in_=xb_tile[:, qt, :])

            # AV1 + LN1, with row-transposes of x1 staggered one qt behind so
            # the PE never waits on the freshest layernorm
            PT = pt_p.tile([P, ST, S], bf16, tag="pt")
            x1b = xb_p.tile([P, ST, D], bf16, tag="xb")
            x1T = xt_p.tile([P, ST, DT, P], bf16, tag="xt", name="x1t")
            for qt in range(ST):
                for kt in range(qt + 1):
                    tp = ps.tile([P, 512], bf16, tag="ps", name="tp")
                    nc.tensor.transpose(out=tp[:, :P],
                                        in_=Pbs[qt][:, kt * P:(kt + 1) * P],
                                        identity=ident)
                    nc.scalar.copy(out=PT[:, kt, qt * P:(qt + 1) * P],
                                   in_=tp[:, :P])
                xpre = xpre_p.tile([P, D], bf16, tag="xpre")
                for nh in range(2):
                    pm = ps.tile([P, 512], f32, tag="ps")
                    for kt in range(qt + 1):
                        nc.tensor.matmul(pm, lhsT=PT[:, kt, qt * P:(qt + 1) * P],
                                         rhs=Vt[:, kt, nh * 512:(nh + 1) * 512],
                                         start=(kt == 0), stop=(kt == qt))
                    nc.vector.scalar_tensor_tensor(
                        out=xpre[:, nh * 512:(nh + 1) * 512], in0=pm,
                        scalar=rinv1[:, qt:qt + 1],
                        in1=x0b[:, qt, nh * 512:(nh + 1) * 512],
                        op0=ALU.mult, op1=ALU.add)
                if qt > 1:
                    transpose_rows(x1b, qt - 2, x1T, "x1")
                layernorm(xpre, x1b[:, qt, :], g1b, b1b)
            transpose_rows(x1b, ST - 2, x1T, "x1")
            emit_k2t()
            emit_v2t()

            # ---- cross attention: Q2, scores2 + softmax, then AV2 ----
            # Q2 columns for the first three row-tiles keep the PE busy while
            # LN1 of the last row-tile drains; its transpose slots in between.
            Q2T = qk_p.tile([P, DT, S], bf16, tag="qk", name="q2t")

            def q2t_range(s0, s1):
                for m in range(DT):
                    w_sb, mb = (wq2a, m * P) if m < 4 else (wq2b, (m - 4) * P)
                    pm = ps.tile([P, 512], f32, tag="ps", name="pm")
                    for k in range(DT):
                        nc.tensor.matmul(pm[:, :s1 - s0],
                                         lhsT=w_sb[:, k, mb:mb + P],
                                         rhs=x1T[:, s0 // P:s1 // P, k, :],
                                         start=(k == 0), stop=(k == DT - 1))
                    nc.scalar.activation(out=Q2T[:, m, s0:s1],
                                         in_=pm[:, :s1 - s0],
                                         func=ACT_F.Identity,
                                         bias=bqall[:, 2, m:m + 1], scale=1.0)

            q2t_range(0, 3 * P)
            transpose_rows(x1b, ST - 1, x1T, "x1")
            q2t_range(3 * P, S)

            P2bs = []
            rinv2 = stat_p.tile([P, ST], f32, tag="rinv")
            for qt in range(ST):
                pm = ps.tile([P, 512], f32, tag="ps")
                for k in range(DT):
                    nc.tensor.matmul(pm[:, :NI],
                                     lhsT=Q2T[:, k, qt * P:(qt + 1) * P],
                                     rhs=K2T[:, k, :NI],
                                     start=(k == 0), stop=(k == DT - 1))
                nmax = stat_p.tile([P, 1], f32, tag="nmax")
                nc.vector.reduce_max(nmax, pm[:, :NI], axis=X, negate=True)
                P2b = pb_p.tile([P, NI_PAD], bf16, tag="pb", name=f"p2b{qt}")
                nc.vector.memset(P2b[:, NI:], 0.0)
                rsum = stat_p.tile([P, 1], f32, tag="rsum")
                nc.scalar.activation(out=P2b[:, :NI], in_=pm[:, :NI],
                                     func=ACT_F.Exp, bias=nmax, scale=1.0,
                                     accum_out=rsum)
                nc.vector.reciprocal(out=rinv2[:, qt:qt + 1], in_=rsum)
                P2bs.append(P2b)

            PT2 = pt_p.tile([P, NIT, S], bf16, tag="pt")
            x2b = xb_p.tile([P, ST, D], bf16, tag="xb")
            x2T = xt_p.tile([P, ST, DT, P], bf16, tag="xt", name="x2t")
            for qt in range(ST):
                for kt in range(NIT):
                    tp = ps.tile([P, 512], bf16, tag="ps", name="tp")
                    nc.tensor.transpose(out=tp[:, :P],
                                        in_=P2bs[qt][:, kt * P:(kt + 1) * P],
                                        identity=ident)
                    nc.scalar.copy(out=PT2[:, kt, qt * P:(qt + 1) * P],
                                   in_=tp[:, :P])
                xpre = xpre_p.tile([P, D], bf16, tag="xpre")
                for nh in range(2):
                    pm = ps.tile([P, 512], f32, tag="ps")
                    for kt in range(NIT):
                        nc.tensor.matmul(pm, lhsT=PT2[:, kt, qt * P:(qt + 1) * P],
                                         rhs=V2t[:, kt, nh * 512:(nh + 1) * 512],
                                         start=(kt == 0), stop=(kt == NIT - 1))
                    nc.vector.scalar_tensor_tensor(
                        out=xpre[:, nh * 512:(nh + 1) * 512], in0=pm,
                        scalar=rinv2[:, qt:qt + 1],
                        in1=x1b[:, qt, nh * 512:(nh + 1) * 512],
                        op0=ALU.mult, op1=ALU.add)
                if qt > 1:
                    transpose_rows(x2b, qt - 2, x2T, "x2")
                layernorm(xpre, x2b[:, qt, :], g2b, b2b)
            transpose_rows(x2b, ST - 2, x2T, "x2")

            # ---- vocab projection, streamed in CN-column chunks ----
            def vocab_group_mm(chunks, widths, offset, qt_list, state=None):
                gw = sum(widths)
                if state is None:
                    bp_bc = bp_p.tile([P, gw], bf16, tag="bp")
                    nc.gpsimd.dma_start(out=bp_bc,
                                        in_=bcast(h_bp, gw, offset=offset))
                    osb = [osb_p.tile([P, gw], bf16, tag="osb",
                                      name=f"osb_{offset}_{q}")
                           for q in range(ST)]
                else:
                    bp_bc, osb = state
                col = 0
                for wp_sb, w in zip(chunks, widths):
                    for qt in qt_list:
                        pm = ps.tile([P, 512], f32, tag="ps")
                        for k in range(DT):
                            nc.tensor.matmul(
                                pm[:, :w], lhsT=x2T[:, qt, k, :],
                                rhs=wp_sb[:, k, :w],
                                start=(k == 0), stop=(k == DT - 1))
                        nc.vector.tensor_tensor(
                            out=osb[qt][:, col:col + w], in0=pm[:, :w],
                            in1=bp_bc[:, col:col + w], op=ALU.add)
                    col += w
                return bp_bc, osb

            def vocab_group_out(osb, offset, gw):
                for qt in range(ST):
                    out_eng = nc.scalar if qt < 2 else nc.gpsimd
                    out_eng.dma_start(
                        out=h_out[qt * P:(qt + 1) * P, offset:offset + gw],
                        in_=osb[qt])

            # group 0: row-tile 3 deferred until its transpose lands
            g0 = [wp_tiles[0], wp_tiles[1]]
            st0 = vocab_group_mm(g0, [CN, CN], 0, [0])
            vocab_group_mm(g0, [CN, CN], 0, [1], state=st0)
            vocab_group_mm(g0, [CN, CN], 0, [2], state=st0)
            transpose_rows(x2b, ST - 1, x2T, "x2")
            vocab_group_mm(g0, [CN, CN], 0, [3], state=st0)
            vocab_group_out(st0[1], 0, GRP * CN)
            for g in range(1, NGRP):
                _, osb = vocab_group_mm(
                    [wp_tiles[g * GRP], wp_tiles[g * GRP + 1]],
                    [CN, CN], g * GRP * CN, list(range(ST)))
                vocab_group_out(osb, g * GRP * CN, GRP * CN)
            _, osbt = vocab_group_mm([wpt_sb], [CTAIL], NFULL * CN,
                                     list(range(ST)))
            vocab_group_out(osbt, NFULL * CN, CTAIL)

    nc.compile()
    return nc


def _tile_sq(w, kt):
    """[K, N] -> [128, K//128, N] contiguous."""
    k, n = w.shape
    assert k == kt * P
    return np.ascontiguousarray(
        w.reshape(kt, P, n).transpose(1, 0, 2)).astype(BF16)


def _prep_inputs(inputs):
    g = lambda name: np.asarray(inputs[name], dtype=np.float32)
    tokens = np.asarray(inputs["tokens"]).astype(np.int64)
    img = g("img_emb")

    # positional encoding (same closed form as the model definition)
    posn = np.arange(S)[:, None].astype(np.float32)
    i = np.arange(0, D, 2).astype(np.float32)
    ang = posn / np.power(10000.0, i / D)
    pos = np.zeros((S, D), dtype=np.float32)
    pos[:, 0::2] = np.sin(ang)
    pos[:, 1::2] = np.cos(ang)

    # embedding gather + positional add on the host (input prep)
    x0 = (g("emb_table")[tokens] + pos[None]).astype(BF16)  # [B, S, D]

    wp = g("Wp")  # [D, V]
    wp_main = np.ascontiguousarray(
        wp[:, :NFULL * CN].reshape(DT, P, NFULL, CN)
        .transpose(2, 1, 0, 3)).astype(BF16)
    wp_tail = _tile_sq(wp[:, NFULL * CN:], DT)
    bp = g("bp").astype(BF16)

    def bias_tiled(b):
        return np.ascontiguousarray(b.reshape(DT, P).T).astype(np.float32)

    shared = {
        "wq1": _tile_sq(g("Wq1") * SCALE, DT),
        "wk1": _tile_sq(g("Wk1"), DT),
        "wv1": _tile_sq(g("Wv1"), DT),
        "wq2": _tile_sq(g("Wq2") * SCALE, DT),
        "wk2": _tile_sq(g("Wk2"), DIT),
        "wv2": _tile_sq(g("Wv2"), DIT),
        "wp": wp_main,
        "wpt": wp_tail,
        "bqs": np.ascontiguousarray(np.stack(
            [bias_tiled(g("bq1") * SCALE), bias_tiled(g("bk1")),
             bias_tiled(g("bq2") * SCALE), bias_tiled(g("bk2"))], axis=1)),
        "bv1": g("bv1").astype(BF16).reshape(1, D),
        "bv2": g("bv2").astype(BF16).reshape(1, D),
        "bp": bp,
        "g1": g("g1").astype(BF16), "b1": g("b1").astype(BF16),
        "g2": g("g2").astype(BF16), "b2": g("b2").astype(BF16),
    }
    in_maps = []
    for c in range(N_CORES):
        m = dict(shared)
        xc = x0[c]  # [S, D] bf16
        m["x0b"] = np.ascontiguousarray(
            xc.reshape(ST, P, D).transpose(1, 0, 2))
        m["x0t"] = np.ascontiguousarray(
            xc.T.reshape(DT, P, S).transpose(1, 0, 2))
        m["img_t"] = np.ascontiguousarray(
            img[c].T.reshape(DIT, P, NI).transpose(1, 0, 2)).astype(BF16)
        in_maps.append(m)
    return in_maps


def _ensure_axon_hooks():
    """bass_utils imports antenv.axon_hooks when BASS_TRACE is set; stub it
    if the module is absent so tracing degrades instead of crashing."""
    try:
        import antenv.axon_hooks  # noqa: F401
    except ImportError:
        import types
        mod = types.ModuleType("antenv.axon_hooks")
        mod.get_axon_ntff_profile_hook = lambda: None
        mod.set_axon_ntff_profile_hook = lambda h: None
        sys.modules["antenv.axon_hooks"] = mod


def kernel(**inputs):
    global LAST_RESULTS
    _ensure_axon_hooks()
    from concourse.bass_utils import run_bass_kernel_spmd

    if "nc" not in _CACHE:
        _CACHE["nc"] = _build_program()
    nc = _CACHE["nc"]

    in_maps = _prep_inputs(inputs)
    res = run_bass_kernel_spmd(nc, in_maps, core_ids=list(range(N_CORES)))
    LAST_RESULTS = res
    out = np.stack([res.results[c]["out"].astype(np.float32)
                    for c in range(N_CORES)])
    return out


# revision 40
# speedup vs baseline: 1.0173x; 1.0108x over previous
"""Trainium2 Bass kernel for an 8-batch image-conditioned decoder layer.

Strategy: pure data-parallel over the batch — core c computes batch element c
end-to-end (causal self-attention, cross-attention over the image tokens, both
layernorms, vocab projection). No collectives.

All matmuls run in bf16 with fp32 PSUM accumulation.  Weights are pre-cast /
pre-tiled on the host into the exact SBUF layouts the TensorEngine consumes
([128 k_inner, k_outer, n]); the vocab projection streams exactly V=32000
columns (62 chunks of 512 + one of 256) from HBM.  The embedding gather and
positional add happen on the host (input prep), shipped both seq-major (x0b,
for residuals) and d-major (x0T, ready for the first projections) so the
TensorEngine starts immediately.  Elementwise work is spread across DVE /
Activation / GpSimd so no single engine serializes the attention phase.
"""

import os
import sys

for _p in ("/opt/trn_rl_repo", "/root/.axon_site/_ro/trn_rl_repo"):
    if os.path.isdir(_p) and _p not in sys.path:
        sys.path.append(_p)

import numpy as np
import ml_dtypes

BF16 = ml_dtypes.bfloat16

# Problem dims (hardcoded per spec)
V, D, DI, S, B, NI = 32000, 1024, 768, 512, 8, 197
EPS = 1e-5
P = 128
ST = S // P          # 4 seq tiles
DT = D // P          # 8 model-dim tiles
DIT = DI // P        # 6 image-dim tiles
NIT = 2              # image tokens: 197 -> 2 partition tiles (128 + 69)
NI_PAD = 256
CN = 512             # vocab chunk width
NFULL = V // CN      # 62 full chunks
CTAIL = V - NFULL * CN   # 256 tail columns
GRP = 2              # full chunks per output strip
NGRP = NFULL // GRP  # 31
N_CORES = 8
SCALE = 1.0 / float(np.sqrt(np.float32(D)))

_CACHE = {}
LAST_RESULTS = None


def _build_program():
    import concourse.bacc as bacc
    import concourse.bass as bass
    import concourse.mybir as mybir
    from concourse.masks import make_identity
    from concourse.tile import TileContext

    f32 = mybir.dt.float32
    bf16 = mybir.dt.bfloat16
    X = mybir.AxisListType.X
    ALU = mybir.AluOpType
    ACT_F = mybir.ActivationFunctionType

    nc = bacc.Bacc("TRN2", target_bir_lowering=False, debug=False,
                   num_devices=N_CORES)

    # ---- I/O ----
    h_x0b = nc.dram_tensor("x0b", [P, ST, D], bf16, kind="ExternalInput")
    h_x0t = nc.dram_tensor("x0t", [P, DT, S], bf16, kind="ExternalInput")
    h_img = nc.dram_tensor("img_t", [P, DIT, NI], bf16, kind="ExternalInput")
    h_wq1 = nc.dram_tensor("wq1", [P, DT, D], bf16, kind="ExternalInput")
    h_wk1 = nc.dram_tensor("wk1", [P, DT, D], bf16, kind="ExternalInput")
    h_wv1 = nc.dram_tensor("wv1", [P, DT, D], bf16, kind="ExternalInput")
    h_wq2 = nc.dram_tensor("wq2", [P, DT, D], bf16, kind="ExternalInput")
    h_wk2 = nc.dram_tensor("wk2", [P, DIT, D], bf16, kind="ExternalInput")
    h_wv2 = nc.dram_tensor("wv2", [P, DIT, D], bf16, kind="ExternalInput")
    h_wp = nc.dram_tensor("wp", [NFULL, P, DT, CN], bf16, kind="ExternalInput")
    h_wpt = nc.dram_tensor("wpt", [P, DT, CTAIL], bf16, kind="ExternalInput")
    h_bqs = nc.dram_tensor("bqs", [P, 4, DT], f32, kind="ExternalInput")
    h_bv1 = nc.dram_tensor("bv1", [1, D], bf16, kind="ExternalInput")
    h_bv2 = nc.dram_tensor("bv2", [1, D], bf16, kind="ExternalInput")
    h_bp = nc.dram_tensor("bp", [V], bf16, kind="ExternalInput")
    h_g1 = nc.dram_tensor("g1", [D], bf16, kind="ExternalInput")
    h_b1 = nc.dram_tensor("b1", [D], bf16, kind="ExternalInput")
    h_g2 = nc.dram_tensor("g2", [D], bf16, kind="ExternalInput")
    h_b2 = nc.dram_tensor("b2", [D], bf16, kind="ExternalInput")
    h_out = nc.dram_tensor("out", [S, V], bf16, kind="ExternalOutput")

    def bcast(handle, n, offset=0):
        ap = handle[:]
        return bass.AP(tensor=ap.tensor, offset=offset, ap=[[0, P], [1, n]])

    with TileContext(nc) as tc:
        import contextlib
        ctx = contextlib.ExitStack()
        with ctx:
            const = ctx.enter_context(tc.tile_pool(name="const", bufs=1))
            xb_p = ctx.enter_context(tc.tile_pool(name="xb", bufs=2))
            xt_p = ctx.enter_context(tc.tile_pool(name="xt", bufs=1))
            qk_p = ctx.enter_context(tc.tile_pool(name="qk", bufs=2))
            v_p = ctx.enter_context(tc.tile_pool(name="vp", bufs=2))
            k2t_p = ctx.enter_context(tc.tile_pool(name="k2t", bufs=1))
            pb_p = ctx.enter_context(tc.tile_pool(name="pb", bufs=4))
            pt_p = ctx.enter_context(tc.tile_pool(name="pt", bufs=1))
            dgl_p = ctx.enter_context(tc.tile_pool(name="dgl", bufs=1))
            xpre_p = ctx.enter_context(tc.tile_pool(name="xpre", bufs=2))
            stat_p = ctx.enter_context(tc.tile_pool(name="stat", bufs=4))
            wts_p = ctx.enter_context(tc.tile_pool(name="wts", bufs=6))
            wp_p = ctx.enter_context(tc.tile_pool(name="wpp", bufs=5))
            bp_p = ctx.enter_context(tc.tile_pool(name="bpp", bufs=2))
            osb_p = ctx.enter_context(tc.tile_pool(name="osb", bufs=5))
            ps = ctx.enter_context(tc.tile_pool(name="ps", bufs=8, space="PSUM"))

            # ---- queue assignments ----
            # All big weights are shipped as 8KB/partition halves so three DMA
            # queues share the early critical bytes and the 6-slot ring never
            # stalls a transfer past its consumer.
            # sync:   x0T, wk1a, wv1a, wp0, wp1, wq2a/b, rest of the wp stream
            # scalar: small biases, wq1a, wk1b, wv1b, bv1/g1/b1, x0b, img,
            #         bv2/g2/b2; later: half the output writes
            # gpsimd: wq1b, wk2a/b, wv2a/b; trimask; bp strips + other outputs
            HD = D // 2
            HK = DT // 2
            x0t_lo = xt_p.tile([P, HK, S], bf16, tag="xtl", name="x0tl")
            nc.sync.dma_start(out=x0t_lo, in_=h_x0t[:, 0:HK, :])
            x0t_hi = xt_p.tile([P, HK, S], bf16, tag="xth", name="x0th")
            nc.scalar.dma_start(out=x0t_hi, in_=h_x0t[:, HK:DT, :])

            def x0T_k(k):
                return (x0t_lo, k) if k < HK else (x0t_hi, k - HK)

            bqall = const.tile([P, 4, DT], f32)
            nc.scalar.dma_start(out=bqall, in_=h_bqs[:])
            QD = D // 4
            wq1q = []
            for qi, eng in enumerate((nc.gpsimd, nc.sync, nc.scalar,
                                      nc.gpsimd)):
                t = wts_p.tile([P, DT, QD], bf16, tag="wts")
                eng.dma_start(out=t, in_=h_wq1[:, :, qi * QD:(qi + 1) * QD])
                wq1q.append(t)
            wk1a = wts_p.tile([P, DT, HD], bf16, tag="wts")
            nc.sync.dma_start(out=wk1a, in_=h_wk1[:, :, 0:HD])
            wk1b = wts_p.tile([P, DT, HD], bf16, tag="wts")
            nc.scalar.dma_start(out=wk1b, in_=h_wk1[:, :, HD:D])
            wv1a = wts_p.tile([P, DT, HD], bf16, tag="wts")
            nc.sync.dma_start(out=wv1a, in_=h_wv1[:, :, 0:HD])
            wv1b = wts_p.tile([P, DT, HD], bf16, tag="wts")
            nc.gpsimd.dma_start(out=wv1b, in_=h_wv1[:, :, HD:D])

            ident = const.tile([P, P], bf16)
            make_identity(nc, ident)
            trimask = const.tile([P, P], f32)
            nc.gpsimd.memset(trimask, 0.0)
            nc.gpsimd.affine_select(
                out=trimask, in_=trimask, compare_op=ALU.is_ge, fill=-1e10,
                base=0, pattern=[[-1, P]], channel_multiplier=1)
            epst = const.tile([P, 1], f32)
            nc.vector.memset(epst, EPS)
            ones_sb = const.tile([P, P], bf16)
            nc.vector.memset(ones_sb, 1.0)
            V2t = v_p.tile([P, NIT, D], bf16, tag="v")
            nc.vector.memset(V2t[:, 1, :], 0.0)

            # single-row bias vectors (consumed via rank-1 ones-row matmuls)
            bv1b = const.tile([P, D], bf16)
            nc.scalar.dma_start(out=bv1b[0:1, :], in_=h_bv1[:])
            bv2b = const.tile([P, D], bf16)
            nc.scalar.dma_start(out=bv2b[0:1, :], in_=h_bv2[:])

            # cross-attn weight halves ride the otherwise-idle gpsimd queue
            wk2a = wts_p.tile([P, DIT, HD], bf16, tag="wts")
            nc.gpsimd.dma_start(out=wk2a, in_=h_wk2[:, :, 0:HD])
            wk2b = wts_p.tile([P, DIT, HD], bf16, tag="wts")
            nc.gpsimd.dma_start(out=wk2b, in_=h_wk2[:, :, HD:D])
            wv2a = wts_p.tile([P, DIT, HD], bf16, tag="wts")
            nc.gpsimd.dma_start(out=wv2a, in_=h_wv2[:, :, 0:HD])
            wv2b = wts_p.tile([P, DIT, HD], bf16, tag="wts")
            nc.gpsimd.dma_start(out=wv2b, in_=h_wv2[:, :, HD:D])

            x0b = xb_p.tile([P, ST, D], bf16, tag="xb", name="x0b")
            nc.scalar.dma_start(out=x0b, in_=h_x0b[:])
            img_sb = const.tile([P, DIT, NI], bf16)
            nc.scalar.dma_start(out=img_sb, in_=h_img[:])

            # layernorm gain/shift broadcasts (needed from ~80us on)
            g1b = const.tile([P, D], bf16)
            b1b = const.tile([P, D], bf16)
            g2b = const.tile([P, D], bf16)
            b2b = const.tile([P, D], bf16)
            for t, h in ((g1b, h_g1), (b1b, h_b1), (g2b, h_g2), (b2b, h_b2)):
                nc.scalar.dma_start(out=t, in_=bcast(h, D))

            # vocab weight stream on sync (wq2 halves slot in mid-stream)
            wp_tiles = []
            for c in range(4):
                wp_sb = wp_p.tile([P, DT, CN], bf16, tag="wp", name=f"wp{c}")
                nc.sync.dma_start(out=wp_sb, in_=h_wp[c])
                wp_tiles.append(wp_sb)
            wpt_sb = const.tile([P, DT, CTAIL], bf16)
            nc.sync.dma_start(out=wpt_sb, in_=h_wpt[:])
            for c in range(4, NFULL):
                wp_sb = wp_p.tile([P, DT, CN], bf16, tag="wp", name=f"wp{c}")
                nc.sync.dma_start(out=wp_sb, in_=h_wp[c])
                wp_tiles.append(wp_sb)

            # ---- projections ----
            def proj_T(w_of_m, b_row, rhs_of_k, name):
                """QT/KT-style: out[P, DT, S] bf16 = (W.T @ x.T) + b, d-partition.
                w_of_m / rhs_of_k map tile indices to (tile, local index)."""
                o = qk_p.tile([P, DT, S], bf16, tag="qk", name=name)
                for m in range(DT):
                    w_sb, mb = w_of_m(m)
                    pm = ps.tile([P, 512], f32, tag="ps", name="pm")
                    for k in range(DT):
                        r_sb, rk = rhs_of_k(k)
                        nc.tensor.matmul(pm,
                                         lhsT=w_sb[:, k, mb:mb + P],
                                         rhs=r_sb[:, rk, :],
                                         start=(k == 0), stop=(k == DT - 1))
                    nc.scalar.activation(out=o[:, m, :], in_=pm,
                                         func=ACT_F.Identity,
                                         bias=bqall[:, b_row, m:m + 1],
                                         scale=1.0)
                return o

            def halved(wa, wb):
                return lambda m: (wa, m * P) if m < 4 else (wb, (m - 4) * P)

            QT = proj_T(lambda m: (wq1q[m // 2], (m % 2) * P), 0, x0T_k, "qt")
            KT = proj_T(halved(wk1a, wk1b), 1, x0T_k, "kt")


            # ---- causal self-attention: scores + softmax (all qt), then AV ----
            # softmax straight out of PSUM: max over the raw scores (safe — the
            # shift cancels in the normalization), mask only the diagonal block.
            Pbs = []
            rinv1 = stat_p.tile([P, ST], f32, tag="rinv")
            for qt in range(ST):
                width = (qt + 1) * P
                pm = ps.tile([P, 512], f32, tag="ps")
                for k in range(DT):
                    nc.tensor.matmul(pm[:, :width],
                                     lhsT=QT[:, k, qt * P:(qt + 1) * P],
                                     rhs=KT[:, k, :width],
                                     start=(k == 0), stop=(k == DT - 1))
                nmax = stat_p.tile([P, 1], f32, tag="nmax")
                nc.vector.reduce_max(nmax, pm[:, :width], axis=X, negate=True)
                diag = dgl_p.tile([P, P], f32, tag="dgl")
                nc.vector.tensor_tensor(out=diag, in0=pm[:, qt * P:width],
                                        in1=trimask, op=ALU.add)
                Pb = pb_p.tile([P, 512], bf16, tag="pb", name=f"pb{qt}")
                rsum = stat_p.tile([P, 1], f32, tag="rsum")
                if qt > 0:
                    rs1 = stat_p.tile([P, 1], f32, tag="rs1")
                    nc.scalar.activation(out=Pb[:, :qt * P], in_=pm[:, :qt * P],
                                         func=ACT_F.Exp, bias=nmax, scale=1.0,
                                         accum_out=rs1)
                    rs2 = stat_p.tile([P, 1], f32, tag="rs2")
                    nc.scalar.activation(out=Pb[:, qt * P:width], in_=diag,
                                         func=ACT_F.Exp, bias=nmax, scale=1.0,
                                         accum_out=rs2)
                    nc.vector.tensor_tensor(out=rsum, in0=rs1, in1=rs2,
                                            op=ALU.add)
                else:
                    nc.scalar.activation(out=Pb[:, :width], in_=diag,
                                         func=ACT_F.Exp, bias=nmax, scale=1.0,
                                         accum_out=rsum)
                nc.vector.reciprocal(out=rinv1[:, qt:qt + 1], in_=rsum)
                Pbs.append(Pb)

            # V projection overlaps the softmax chain above
            Vt = v_p.tile([P, ST, D], bf16, tag="v")
            for a in range(ST):
                for nh in range(2):
                    wv = wv1a if nh == 0 else wv1b
                    pm = ps.tile([P, 512], f32, tag="ps")
                    for k in range(DT):
                        xkt, xk = x0T_k(k)
                        nc.tensor.matmul(
                            pm, lhsT=xkt[:, xk, a * P:(a + 1) * P],
                            rhs=wv[:, k, :],
                            start=(k == 0), stop=False)
                    nc.tensor.matmul(
                        pm, lhsT=ones_sb[0:1, :],
                        rhs=bv1b[0:1, nh * 512:(nh + 1) * 512],
                        start=False, stop=True)
                    nc.scalar.copy(out=Vt[:, a, nh * 512:(nh + 1) * 512],
                                   in_=pm)

            # wq2 triggers emitted here: their ring slots (ex-wv1) are free by
            # the time the Act queue reaches them, so nothing stalls
            wq2a = wts_p.tile([P, DT, HD], bf16, tag="wts")
            nc.scalar.dma_start(out=wq2a, in_=h_wq2[:, :, 0:HD])
            wq2b = wts_p.tile([P, DT, HD], bf16, tag="wts")
            nc.scalar.dma_start(out=wq2b, in_=h_wq2[:, :, HD:D])

            def layernorm(xpre, out_sl, gb, bb):
                """xpre [P, D] f32 -> out_sl [P, D] bf16 (normalized * g + b)."""
                stats = stat_p.tile([P, 2, 6], f32, tag="bnst")
                for sg in range(2):
                    nc.vector.bn_stats(out=stats[:, sg, :],
                                       in_=xpre[:, sg * 512:(sg + 1) * 512])
                mv = stat_p.tile([P, 2], f32, tag="bnmv")
                nc.vector.bn_aggr(out=mv, in_=stats)
                rstd = stat_p.tile([P, 1], f32, tag="rstd")
                nc.scalar.activation(out=rstd, in_=mv[:, 1:2], func=ACT_F.Sqrt,
                                     bias=epst, scale=1.0)
                nc.vector.reciprocal(out=rstd, in_=rstd)
                nmr = stat_p.tile([P, 1], f32, tag="nmr")
                nc.vector.tensor_tensor(out=nmr, in0=mv[:, 0:1], in1=rstd,
                                        op=ALU.mult)
                nc.scalar.mul(nmr, nmr, -1.0)
                nc.scalar.activation(out=xpre, in_=xpre, func=ACT_F.Identity,
                                     bias=nmr, scale=rstd)
                nc.vector.tensor_tensor(out=xpre, in0=xpre, in1=gb,
                                        op=ALU.mult)
                nc.vector.tensor_tensor(out=out_sl, in0=xpre, in1=bb,
                                        op=ALU.add)

            # ---- cross-attn K2/V2 (emitted after AV1 so this independent PE
            # work fills the layernorm-chain window) ----
            K2T = k2t_p.tile([P, DT, NI_PAD], bf16, tag="k2t")

            def emit_k2t():
                for m in range(DT):
                    wk2, mb = (wk2a, m * P) if m < 4 else (wk2b, (m - 4) * P)
                    pm = ps.tile([P, 512], f32, tag="ps")
                    for k in range(DIT):
                        nc.tensor.matmul(pm[:, :NI],
                                         lhsT=wk2[:, k, mb:mb + P],
                                         rhs=img_sb[:, k, :],
                                         start=(k == 0), stop=(k == DIT - 1))
                    nc.scalar.activation(out=K2T[:, m, :NI], in_=pm[:, :NI],
                                         func=ACT_F.Identity,
                                         bias=bqall[:, 3, m:m + 1], scale=1.0)

            def emit_v2t():
              for a in range(NIT):
                  pa = P if a == 0 else NI - P
                  for nh in range(2):
                      wv2 = wv2a if nh == 0 else wv2b
                      pm = ps.tile([P, 512], f32, tag="ps")
                      for k in range(DIT):
                          nc.tensor.matmul(
                              pm[:pa, :], lhsT=img_sb[:, k, a * P:a * P + pa],
                              rhs=wv2[:, k, :],
                              start=(k == 0), stop=False)
                      nc.tensor.matmul(
                          pm[:pa, :], lhsT=ones_sb[0:1, :pa],
                          rhs=bv2b[0:1, nh * 512:(nh + 1) * 512],
                          start=False, stop=True)
                      nc.scalar.copy(out=V2t[:pa, a, nh * 512:(nh + 1) * 512],
                                     in_=pm[:pa, :])

            def transpose_rows(xb_tile, qt, xt, tag):
                """XBAR-transpose row-tile qt of [P, ST, D] into the qt-major
                d-transposed tile xt [P, ST, DT, P] — zero PE/Act cost."""
                nc.scalar.dma_start_transpose(out=xt[:, qt, :, :],
                                               in_=xb_tile[:, qt, :])

            # AV1 + LN1, with row-transposes of x1 staggered one qt behind so
            # the PE never waits on the freshest layernorm
            PT = pt_p.tile([P, ST, S], bf16, tag="pt")
            x1b = xb_p.tile([P, ST, D], bf16, tag="xb")
            x1T = xt_p.tile([P, ST, DT, P], bf16, tag="xt", name="x1t")
            for qt in range(ST):
                for kt in range(qt + 1):
                    tp = ps.tile([P, 512], bf16, tag="ps", name="tp")
                    nc.tensor.transpose(out=tp[:, :P],
                                        in_=Pbs[qt][:, kt * P:(kt + 1) * P],
                                        identity=ident)
                    nc.scalar.copy(out=PT[:, kt, qt * P:(qt + 1) * P],
                                   in_=tp[:, :P])
                xpre = xpre_p.tile([P, D], bf16, tag="xpre")
                for nh in range(2):
                    pm = ps.tile([P, 512], f32, tag="ps")
                    for kt in range(qt + 1):
                        nc.tensor.matmul(pm, lhsT=PT[:, kt, qt * P:(qt + 1) * P],
                                         rhs=Vt[:, kt, nh * 512:(nh + 1) * 512],
                                         start=(kt == 0), stop=(kt == qt))
                    nc.vector.scalar_tensor_tensor(
                        out=xpre[:, nh * 512:(nh + 1) * 512], in0=pm,
                        scalar=rinv1[:, qt:qt + 1],
                        in1=x0b[:, qt, nh * 512:(nh + 1) * 512],
                        op0=ALU.mult, op1=ALU.add)
                if qt > 1:
                    transpose_rows(x1b, qt - 2, x1T, "x1")
                layernorm(xpre, x1b[:, qt, :], g1b, b1b)
            transpose_rows(x1b, ST - 2, x1T, "x1")
            emit_k2t()
            emit_v2t()

            # ---- cross attention: Q2, scores2 + softmax, then AV2 ----
            # Q2 columns for the first three row-tiles keep the PE busy while
            # LN1 of the last row-tile drains; its transpose slots in between.
            Q2T = qk_p.tile([P, DT, S], bf16, tag="qk", name="q2t")

            def q2t_range(s0, s1):
                for m in range(DT):
                    w_sb, mb = (wq2a, m * P) if m < 4 else (wq2b, (m - 4) * P)
                    pm = ps.tile([P, 512], f32, tag="ps", name="pm")
                    for k in range(DT):
                        nc.tensor.matmul(pm[:, :s1 - s0],
                                         lhsT=w_sb[:, k, mb:mb + P],
                                         rhs=x1T[:, s0 // P:s1 // P, k, :],
                                         start=(k == 0), stop=(k == DT - 1))
                    nc.scalar.activation(out=Q2T[:, m, s0:s1],
                                         in_=pm[:, :s1 - s0],
                                         func=ACT_F.Identity,
                                         bias=bqall[:, 2, m:m + 1], scale=1.0)

            P2bs = [None] * ST
            rinv2 = stat_p.tile([P, ST], f32, tag="rinv")

            def scores2_qt(qt):
                pm = ps.tile([P, 512], f32, tag="ps")
                for k in range(DT):
                    nc.tensor.matmul(pm[:, :NI],
                                     lhsT=Q2T[:, k, qt * P:(qt + 1) * P],
                                     rhs=K2T[:, k, :NI],
                                     start=(k == 0), stop=(k == DT - 1))
                nmax = stat_p.tile([P, 1], f32, tag="nmax")
                nc.vector.reduce_max(nmax, pm[:, :NI], axis=X, negate=True)
                P2b = pb_p.tile([P, NI_PAD], bf16, tag="pb", name=f"p2b{qt}")
                nc.vector.memset(P2b[:, NI:], 0.0)
                rsum = stat_p.tile([P, 1], f32, tag="rsum")
                nc.scalar.activation(out=P2b[:, :NI], in_=pm[:, :NI],
                                     func=ACT_F.Exp, bias=nmax, scale=1.0,
                                     accum_out=rsum)
                nc.vector.reciprocal(out=rinv2[:, qt:qt + 1], in_=rsum)
                P2bs[qt] = P2b

            # Q2 columns for row-tile 0 first: its cross-attn scores/softmax
            # chain starts while LN1 of the later row-tiles still drains
            q2t_range(0, P)
            scores2_qt(0)
            q2t_range(P, 3 * P)
            transpose_rows(x1b, ST - 1, x1T, "x1")
            q2t_range(3 * P, S)
            for qt in (1, 2, 3):
                scores2_qt(qt)

            PT2 = pt_p.tile([P, NIT, S], bf16, tag="pt")
            x2b = xb_p.tile([P, ST, D], bf16, tag="xb")
            x2T = xt_p.tile([P, ST, DT, P], bf16, tag="xt", name="x2t")
            for qt in range(ST):
                for kt in range(NIT):
                    tp = ps.tile([P, 512], bf16, tag="ps", name="tp")
                    nc.tensor.transpose(out=tp[:, :P],
                                        in_=P2bs[qt][:, kt * P:(kt + 1) * P],
                                        identity=ident)
                    nc.scalar.copy(out=PT2[:, kt, qt * P:(qt + 1) * P],
                                   in_=tp[:, :P])
                xpre = xpre_p.tile([P, D], bf16, tag="xpre")
                for nh in range(2):
                    pm = ps.tile([P, 512], f32, tag="ps")
                    for kt in range(NIT):
                        nc.tensor.matmul(pm, lhsT=PT2[:, kt, qt * P:(qt + 1) * P],
                                         rhs=V2t[:, kt, nh * 512:(nh + 1) * 512],
                                         start=(kt == 0), stop=(kt == NIT - 1))
                    nc.vector.scalar_tensor_tensor(
                        out=xpre[:, nh * 512:(nh + 1) * 512], in0=pm,
                        scalar=rinv2[:, qt:qt + 1],
                        in1=x1b[:, qt, nh * 512:(nh + 1) * 512],
                        op0=ALU.mult, op1=ALU.add)
                if qt > 1:
                    transpose_rows(x2b, qt - 2, x2T, "x2")
                layernorm(xpre, x2b[:, qt, :], g2b, b2b)
            transpose_rows(x2b, ST - 2, x2T, "x2")

            # ---- vocab projection, streamed in CN-column chunks ----
            def vocab_group_mm(chunks, widths, offset, qt_list, state=None):
                gw = sum(widths)
                if state is None:
                    bp_bc = bp_p.tile([P, gw], bf16, tag="bp")
                    nc.gpsimd.dma_start(out=bp_bc,
                                        in_=bcast(h_bp, gw, offset=offset))
                    osb = [osb_p.tile([P, gw], bf16, tag="osb",
                                      name=f"osb_{offset}_{q}")
                           for q in range(ST)]
                else:
                    bp_bc, osb = state
                col = 0
                for wp_sb, w in zip(chunks, widths):
                    for qt in qt_list:
                        pm = ps.tile([P, 512], f32, tag="ps")
                        for k in range(DT):
                            nc.tensor.matmul(
                                pm[:, :w], lhsT=x2T[:, qt, k, :],
                                rhs=wp_sb[:, k, :w],
                                start=(k == 0), stop=(k == DT - 1))
                        nc.vector.tensor_tensor(
                            out=osb[qt][:, col:col + w], in0=pm[:, :w],
                            in1=bp_bc[:, col:col + w], op=ALU.add)
                    col += w
                return bp_bc, osb

            def vocab_group_out(osb, offset, gw):
                for qt in range(ST):
                    out_eng = nc.scalar if qt < 2 else nc.gpsimd
                    out_eng.dma_start(
                        out=h_out[qt * P:(qt + 1) * P, offset:offset + gw],
                        in_=osb[qt])

            # group 0: row-tile 3 deferred until its transpose lands
            g0 = [wp_tiles[0], wp_tiles[1]]
            st0 = vocab_group_mm(g0, [CN, CN], 0, [0])
            vocab_group_mm(g0, [CN, CN], 0, [1], state=st0)
            vocab_group_mm(g0, [CN, CN], 0, [2], state=st0)
            transpose_rows(x2b, ST - 1, x2T, "x2")
            vocab_group_mm(g0, [CN, CN], 0, [3], state=st0)
            vocab_group_out(st0[1], 0, GRP * CN)
            for g in range(1, NGRP):
                _, osb = vocab_group_mm(
                    [wp_tiles[g * GRP], wp_tiles[g * GRP + 1]],
                    [CN, CN], g * GRP * CN, list(range(ST)))
                vocab_group_out(osb, g * GRP * CN, GRP * CN)
            stt_ = vocab_group_mm([wpt_sb], [CTAIL], NFULL * CN, [0])
            nc.scalar.dma_start(out=h_out[0:P, NFULL * CN:V], in_=stt_[1][0])
            for q in (1, 2, 3):
                vocab_group_mm([wpt_sb], [CTAIL], NFULL * CN, [q], state=stt_)
                out_eng = nc.scalar if q < 2 else nc.gpsimd
                out_eng.dma_start(out=h_out[q * P:(q + 1) * P, NFULL * CN:V],
                                  in_=stt_[1][q])

    nc.compile()
    return nc


def _tile_sq(w, kt):
    """[K, N] -> [128, K//128, N] contiguous."""
    k, n = w.shape
    assert k == kt * P
    return np.ascontiguousarray(
        w.reshape(kt, P, n).transpose(1, 0, 2)).astype(BF16)


def _prep_inputs(inputs):
    g = lambda name: np.asarray(inputs[name], dtype=np.float32)
    tokens = np.asarray(inputs["tokens"]).astype(np.int64)
    img = g("img_emb")

    # positional encoding (same closed form as the model definition)
    posn = np.arange(S)[:, None].astype(np.float32)
    i = np.arange(0, D, 2).astype(np.float32)
    ang = posn / np.power(10000.0, i / D)
    pos = np.zeros((S, D), dtype=np.float32)
    pos[:, 0::2] = np.sin(ang)
    pos[:, 1::2] = np.cos(ang)

    # embedding gather + positional add on the host (input prep)
    x0 = (g("emb_table")[tokens] + pos[None]).astype(BF16)  # [B, S, D]

    wp = g("Wp")  # [D, V]
    wp_main = np.ascontiguousarray(
        wp[:, :NFULL * CN].reshape(DT, P, NFULL, CN)
        .transpose(2, 1, 0, 3)).astype(BF16)
    wp_tail = _tile_sq(wp[:, NFULL * CN:], DT)
    bp = g("bp").astype(BF16)

    def bias_tiled(b):
        return np.ascontiguousarray(b.reshape(DT, P).T).astype(np.float32)

    shared = {
        "wq1": _tile_sq(g("Wq1") * SCALE, DT),
        "wk1": _tile_sq(g("Wk1"), DT),
        "wv1": _tile_sq(g("Wv1"), DT),
        "wq2": _tile_sq(g("Wq2") * SCALE, DT),
        "wk2": _tile_sq(g("Wk2"), DIT),
        "wv2": _tile_sq(g("Wv2"), DIT),
        "wp": wp_main,
        "wpt": wp_tail,
        "bqs": np.ascontiguousarray(np.stack(
            [bias_tiled(g("bq1") * SCALE), bias_tiled(g("bk1")),
             bias_tiled(g("bq2") * SCALE), bias_tiled(g("bk2"))], axis=1)),
        "bv1": g("bv1").astype(BF16).reshape(1, D),
        "bv2": g("bv2").astype(BF16).reshape(1, D),
        "bp": bp,
        "g1": g("g1").astype(BF16), "b1": g("b1").astype(BF16),
        "g2": g("g2").astype(BF16), "b2": g("b2").astype(BF16),
    }
    in_maps = []
    for c in range(N_CORES):
        m = dict(shared)
        xc = x0[c]  # [S, D] bf16
        m["x0b"] = np.ascontiguousarray(
            xc.reshape(ST, P, D).transpose(1, 0, 2))
        m["x0t"] = np.ascontiguousarray(
            xc.T.reshape(DT, P, S).transpose(1, 0, 2))
        m["img_t"] = np.ascontiguousarray(
            img[c].T.reshape(DIT, P, NI).transpose(1, 0, 2)).astype(BF16)
        in_maps.append(m)
    return in_maps


def _ensure_axon_hooks():
    """bass_utils imports antenv.axon_hooks when BASS_TRACE is set; stub it
    if the module is absent so tracing degrades instead of crashing."""
    try:
        import antenv.axon_hooks  # noqa: F401
    except ImportError:
        import types
        mod = types.ModuleType("antenv.axon_hooks")
        mod.get_axon_ntff_profile_hook = lambda: None
        mod.set_axon_ntff_profile_hook = lambda h: None
        sys.modules["antenv.axon_hooks"] = mod


def kernel(**inputs):
    global LAST_RESULTS
    _ensure_axon_hooks()
    from concourse.bass_utils import run_bass_kernel_spmd

    if "nc" not in _CACHE:
        _CACHE["nc"] = _build_program()
    nc = _CACHE["nc"]

    in_maps = _prep_inputs(inputs)
    res = run_bass_kernel_spmd(nc, in_maps, core_ids=list(range(N_CORES)))
    LAST_RESULTS = res
    out = np.stack([res.results[c]["out"].astype(np.float32)
                    for c in range(N_CORES)])
    return out
